# revision 29
# baseline (speedup 1.0000x reference)
"""Multi-head attention (B=2, S=2048, D=1024, 16 heads) on 8 Trainium2 cores.

Sharding: core c = 4*b + g handles batch b and heads [4g, 4g+4) — data
parallel over B, tensor parallel over heads (Wq/Wk/Wv column slices, Wo row
slices). The attention matrix never crosses cores.

Per-core device program (everything kept in transposed layouts so the PE
contracts over partitions and softmax denominators ride a ones-column):

  QhT/KhT [256, S] = Wslice.T @ x.T          (x.T streamed from DRAM)
  Vh      [S, 260] = x.T-chunks @ Wv slice   (natural layout, + ones col/head)
  scoresT [k, q]   = KhT-head.T-slices @ QhT-head-slices       (K=64)
  expT             = exp(scoresT / 8)        (ACT, psum->sbuf)
  ctxT_ext[65, q]  = [Vh_h | 1].T @ expT     (rows 0-63 ctx, row 64 = denom)
  rb[128, q]       = ones.T @ (1/denom)      (PE outer-product broadcast)
  attT             = expT * rb + (-1e9*mask)[k]  -> DMA out
  out_partial      = ctxT.T @ Wo_rows            -> DMA out

Matmuls run in float32r (hardware-rounded fp32: ~1.5e-4 rel err, 4x the
fp32 matmul rate). The host transposes attT back (stride view), sums the 4
partial outputs per batch, and adds the analytic mask-context correction
(-1e9 * mask @ (v@Wv + bv) @ Wo, constant over q) plus biases.
"""

import numpy as np

import concourse.bass as bass
import concourse.mybir as mybir
import concourse.tile as tile
from concourse import bacc
from concourse.bass_utils import run_bass_kernel_spmd
from concourse.masks import make_identity

B, S, D = 2, 2048, 1024
H_ALL, DEPTH = 16, 64
NCORES = 8
HG = 4                      # heads per core
GD = HG * DEPTH             # 256 projected dims per core
P = 128
NB = 512                    # q-block / matmul free-dim size
F32 = mybir.dt.float32
F32R = mybir.dt.float32r
AF = mybir.ActivationFunctionType

KC = D // P                 # 8 contraction chunks for projections
MQ = S // P                 # 16 seq chunks of 128
QB = S // NB                # 4 q-blocks of 512

_BUILT = {}
CFG = {}


def _build():
    nc = bacc.Bacc("TRN2", target_bir_lowering=False, debug=False)

    qT = nc.dram_tensor("qT", [D, S], F32, kind="ExternalInput").ap()
    kT = nc.dram_tensor("kT", [D, S], F32, kind="ExternalInput").ap()
    vT = nc.dram_tensor("vT", [D, S], F32, kind="ExternalInput").ap()
    wq = nc.dram_tensor("wq", [D, GD], F32, kind="ExternalInput").ap()
    wk = nc.dram_tensor("wk", [D, GD], F32, kind="ExternalInput").ap()
    wv = nc.dram_tensor("wv", [D, GD], F32, kind="ExternalInput").ap()
    wo = nc.dram_tensor("wo", [GD, D], F32, kind="ExternalInput").ap()
    bq = nc.dram_tensor("bq", [GD], F32, kind="ExternalInput").ap()
    bk = nc.dram_tensor("bk", [GD], F32, kind="ExternalInput").ap()
    bv = nc.dram_tensor("bv", [GD], F32, kind="ExternalInput").ap()
    mneg = nc.dram_tensor("mneg", [S], F32, kind="ExternalInput").ap()
    att_t = nc.dram_tensor("att_t", [HG, S, S], F32, kind="ExternalOutput").ap()
    out_p = nc.dram_tensor("out_p", [S, D], F32, kind="ExternalOutput").ap()

    with tile.TileContext(nc) as tc:
        _body(tc, qT, kT, vT, wq, wk, wv, wo, bq, bk, bv, mneg, att_t, out_p)
    nc.compile()
    return nc


def _body(tc, qT, kT, vT, wq, wk, wv, wo, bq, bk, bv, mneg, att_t, out_p):
    nc = tc.nc

    with tc.tile_pool(name="persist", bufs=1) as pp:
        # ---- persistent constants (loaded once) ----
        # weights, rearranged so contraction chunk kc sits at cols [kc*GD, ...)
        w_sb = {}
        with tc.tile_pool(name="wldp", bufs=2) as wldp:
            for nm, src in (("wq", wq), ("wk", wk), ("wv", wv)):
                t = pp.tile([P, KC * GD], F32R, tag=f"{nm}r", name=f"{nm}r")
                lt = wldp.tile([P, KC * GD], F32, tag="wld", name=f"{nm}ld")
                nc.sync.dma_start(
                    lt[:].rearrange("p (kc j) -> p kc j", kc=KC),
                    src.rearrange("(kc p) j -> p kc j", p=P),
                )
                nc.vector.tensor_copy(t[:], lt[:])
                w_sb[nm] = t
            wo_sb = pp.tile([P, 2 * D], F32R, tag="wor")
            wo_ld = wldp.tile([P, 2 * D], F32, tag="wld", name="wold")
            nc.sync.dma_start(
                wo_ld[:].rearrange("p (dc j) -> p dc j", dc=2),
                wo.rearrange("(dc p) j -> p dc j", p=P),
            )
            nc.vector.tensor_copy(wo_sb[:], wo_ld[:])

        bq_sb = pp.tile([P, 2], F32, tag="bq")
        nc.sync.dma_start(bq_sb[:], bq.rearrange("(m p) -> p m", p=P))
        bk_sb = pp.tile([P, 2], F32, tag="bk")
        nc.sync.dma_start(bk_sb[:], bk.rearrange("(m p) -> p m", p=P))
        bv_sb = pp.tile([P, 2], F32, tag="bv")
        nc.sync.dma_start(bv_sb[:], bv.rearrange("(m p) -> p m", p=P))
        mn_sb = pp.tile([P, MQ], F32, tag="mn")
        nc.sync.dma_start(mn_sb[:], mneg.rearrange("(a p) -> p a", p=P))
        onec_f = pp.tile([P, 1], F32, tag="onecf")
        nc.vector.memset(onec_f[:], 1.0)

        # ---- persistent activations ----
        QhT = [pp.tile([P, S], F32R, tag=f"QhT{m}", name=f"QhT{m}") for m in range(2)]
        KhT = [pp.tile([P, S], F32R, tag=f"KhT{m}", name=f"KhT{m}") for m in range(2)]
        Vh = [
            pp.tile([P, HG * 65], F32R, tag=f"Vh{m}", name=f"Vh{m}")
            for m in range(MQ)
        ]
        ctxT = [
            pp.tile([P, S], F32R, tag=f"ctxT{m}", name=f"ctxT{m}") for m in range(2)
        ]

        # ================= Phase A: projections =================
        with tc.tile_pool(name="stream", bufs=3) as sp, \
             tc.tile_pool(name="psA", bufs=1, space="PSUM") as psA:
            # V -> transposed proj (single pass over vT), then PE-transpose
            # into natural layout [S, GD] with a ones column per head
            VhT = [sp.tile([P, S], F32R, tag=f"VhT{m}", name=f"VhT{m}", bufs=1) for m in range(2)]
            psums = [
                psA.tile([P, NB], F32, tag=f"psA{i}", name=f"ps_wv_{i}")
                for i in range(8)
            ]
            for kc in range(KC):
                xs = sp.tile([P, S], F32, tag="xs")
                nc.sync.dma_start(xs[:], vT[kc * P : (kc + 1) * P, :])
                xr = sp.tile([P, S], F32R, tag="xr")
                nc.vector.tensor_copy(xr[:], xs[:])
                for m in range(2):
                    for n in range(QB):
                        nc.tensor.matmul(
                            psums[m * QB + n][:],
                            w_sb["wv"][:, kc * GD + m * P : kc * GD + (m + 1) * P],
                            xr[:, n * NB : (n + 1) * NB],
                            start=(kc == 0),
                            stop=(kc == KC - 1),
                        )
            for m in range(2):
                for n in range(QB):
                    nc.scalar.activation(
                        VhT[m][:, n * NB : (n + 1) * NB],
                        psums[m * QB + n][:],
                        AF.Identity,
                        bias=bv_sb[:, m : m + 1],
                    )
            ident_f = sp.tile([P, P], F32, tag="identf", bufs=1)
            make_identity(nc, ident_f[:])
            ident = sp.tile([P, P], F32R, tag="ident", bufs=1)
            nc.vector.tensor_copy(ident[:], ident_f[:])
            for m16 in range(MQ):
                for t2 in range(2):
                    pst = psA.tile(
                        [P, P], F32R, tag=f"psA{t2}", name=f"ps_tr_{m16}_{t2}"
                    )
                    nc.tensor.transpose(
                        pst[:], VhT[t2][:, m16 * P : (m16 + 1) * P], ident[:]
                    )
                    for hh in range(2):
                        h = 2 * t2 + hh
                        nc.scalar.activation(
                            Vh[m16][:, 65 * h : 65 * h + 64],
                            pst[:, 64 * hh : 64 * hh + 64],
                            AF.Identity,
                        )
                for h in range(HG):
                    nc.vector.tensor_copy(
                        Vh[m16][:, 65 * h + 64 : 65 * h + 65], onec_f[:]
                    )

            # Q and K -> transposed layout [GD, S]
            for src, wname, bias_sb, out_tiles in (
                (qT, "wq", bq_sb, QhT),
                (kT, "wk", bk_sb, KhT),
            ):
                psums = [
                    psA.tile([P, NB], F32, tag=f"psA{i}", name=f"ps_{wname}_{i}")
                    for i in range(8)
                ]
                for kc in range(KC):
                    xs = sp.tile([P, S], F32, tag="xs")
                    nc.sync.dma_start(xs[:], src[kc * P : (kc + 1) * P, :])
                    xr = sp.tile([P, S], F32R, tag="xr")
                    nc.vector.tensor_copy(xr[:], xs[:])
                    for m in range(2):
                        for n in range(QB):
                            nc.tensor.matmul(
                                psums[m * QB + n][:],
                                w_sb[wname][:, kc * GD + m * P : kc * GD + (m + 1) * P],
                                xr[:, n * NB : (n + 1) * NB],
                                start=(kc == 0),
                                stop=(kc == KC - 1),
                            )
                for m in range(2):
                    for n in range(QB):
                        nc.scalar.activation(
                            out_tiles[m][:, n * NB : (n + 1) * NB],
                            psums[m * QB + n][:],
                            AF.Identity,
                            bias=bias_sb[:, m : m + 1],
                        )
        # ================= Phase B: attention + fused output proj ==========
        # q-block outer, head inner: after all 4 heads finish a q-block its
        # ctxT columns are final, so the output projection for that q-range
        # runs inside phase B (overlapped). Normalize muls and mask adds are
        # split between GPSIMD (otherwise idle) and DVE.
        GP_MULS = CFG.get("gp_muls", 0)    # per block on gpsimd, rest DVE
        GP_MASKS = CFG.get("gp_masks", 12)  # per block on gpsimd, rest DVE
        with tc.tile_pool(name="attn", bufs=1) as ap_, \
             tc.tile_pool(name="outp", bufs=1) as op_, \
             tc.tile_pool(name="psB", bufs=1, space="PSUM") as psB:
            for qb in range(QB):
                qsl = slice(qb * NB, (qb + 1) * NB)
                for h in range(HG):
                    t2, o64 = h // 2, 64 * (h % 2)
                    psc = psB.tile([65, NB], F32, tag="psc", bufs=CFG.get("psc_bufs", 2), name=f"psc{h}_{qb}")
                    exps = []
                    for kc2 in range(MQ):
                        pss = psB.tile(
                            [P, NB], F32, tag="pss", bufs=CFG.get("pss_bufs", 4),
                            name=f"pss{h}_{qb}_{kc2}"
                        )
                        nc.tensor.matmul(
                            pss[:],
                            KhT[t2][o64 : o64 + 64, kc2 * P : (kc2 + 1) * P],
                            QhT[t2][o64 : o64 + 64, qsl],
                            start=True,
                            stop=True,
                        )
                        et = ap_.tile(
                            [P, NB], F32R, tag=f"exp{kc2}", bufs=CFG.get("exp3", 0) and (3 if kc2 < CFG.get("exp3", 0) else 2) or 2,
                            name=f"exp{h}_{qb}_{kc2}",
                        )
                        nc.scalar.activation(et[:], pss[:], AF.Exp, scale=0.125)
                        exps.append(et)
                        nc.tensor.matmul(
                            psc[:],
                            Vh[kc2][:, 65 * h : 65 * h + 65],
                            et[:],
                            start=(kc2 == 0),
                            stop=(kc2 == MQ - 1),
                        )
                    rec = ap_.tile([1, NB], F32R, tag="rec", bufs=2, name=f"rec{h}_{qb}")
                    with nc.allow_low_precision(reason="f32r ~ f32 for recip"):
                        nc.vector.reciprocal(rec[:], psc[64:65, :])
                    rb = ap_.tile([P, NB], F32R, tag="rb", bufs=CFG.get("rb_bufs", 2), name=f"rb{h}_{qb}")
                    nc.gpsimd.partition_broadcast(rb[:], rec[0:1, :])
                    nc.vector.tensor_mul(
                        ctxT[t2][o64 : o64 + 64, qsl], psc[0:64, :], rb[0:64, :]
                    )
                    for kc2 in range(MQ):
                        et = exps[kc2]
                        st = ap_.tile(
                            [P, NB], F32, tag="stage", bufs=CFG.get("stage_bufs", 12),
                            name=f"st{h}_{qb}_{kc2}",
                        )
                        mul_gp = kc2 < GP_MULS
                        mask_gp = GP_MULS <= kc2 < GP_MULS + GP_MASKS
                        if mul_gp:
                            nc.gpsimd.tensor_mul(st[:], et[:], rb[:])
                        else:
                            nc.vector.tensor_mul(st[:], et[:], rb[:])
                        if mask_gp:
                            nc.gpsimd.tensor_scalar_add(
                                st[:], st[:], mn_sb[:, kc2 : kc2 + 1]
                            )
                        else:
                            nc.vector.tensor_scalar_add(
                                st[:], st[:], mn_sb[:, kc2 : kc2 + 1]
                            )
                        nc.sync.dma_start(
                            att_t[h, kc2 * P : (kc2 + 1) * P, qsl], st[:]
                        )
                # output projection for this q-block (ctxT columns now final)
                for qc in range(4 * qb, 4 * (qb + 1)):
                    for nn in range(2):
                        pso = psB.tile(
                            [P, NB], F32, tag="pso", bufs=CFG.get("pso_bufs", 2), name=f"pso{qc}_{nn}"
                        )
                        for dc in range(2):
                            nc.tensor.matmul(
                                pso[:],
                                ctxT[dc][:, qc * P : (qc + 1) * P],
                                wo_sb[:, dc * D + nn * NB : dc * D + (nn + 1) * NB],
                                start=(dc == 0),
                                stop=(dc == 1),
                            )
                        ot = op_.tile([P, NB], F32, tag="ot", bufs=4, name=f"ot{qc}_{nn}")
                        nc.scalar.activation(ot[:], pso[:], AF.Identity)
                        nc.sync.dma_start(
                            out_p[qc * P : (qc + 1) * P, nn * NB : (nn + 1) * NB], ot[:]
                        )

def get_nc():
    if "nc" not in _BUILT:
        _BUILT["nc"] = _build()
    return _BUILT["nc"]


def make_in_maps(q, k, v, mask, Wq, bq, Wk, bk, Wv, bv, Wo, bo):
    q, k, v = (np.asarray(x, np.float32) for x in (q, k, v))
    in_maps = []
    for c in range(NCORES):
        b, g = divmod(c, HG)
        sl = slice(GD * g, GD * (g + 1))
        in_maps.append(
            {
                "qT": np.ascontiguousarray(q[b].T),
                "kT": np.ascontiguousarray(k[b].T),
                "vT": np.ascontiguousarray(v[b].T),
                "wq": np.ascontiguousarray(Wq[:, sl], dtype=np.float32),
                "wk": np.ascontiguousarray(Wk[:, sl], dtype=np.float32),
                "wv": np.ascontiguousarray(Wv[:, sl], dtype=np.float32),
                "wo": np.ascontiguousarray(Wo[sl, :], dtype=np.float32),
                "bq": np.ascontiguousarray(bq[sl], dtype=np.float32),
                "bk": np.ascontiguousarray(bk[sl], dtype=np.float32),
                "bv": np.ascontiguousarray(bv[sl], dtype=np.float32),
                "mneg": np.ascontiguousarray(
                    mask[b, 0, 0].astype(np.float32) * np.float32(-1e9)
                ),
            }
        )
    return in_maps


def assemble(results, q, k, v, mask, Wv, bv, Wo, bo):
    out = np.empty((B, S, D), np.float32)
    attT = np.empty((B, H_ALL, S, S), np.float32)
    for b in range(B):
        acc = None
        for g in range(HG):
            r = results[HG * b + g]
            acc = r["out_p"].copy() if acc is None else acc + r["out_p"]
            attT[b, HG * g : HG * (g + 1)] = r["att_t"]
        mrow = np.asarray(mask, np.float32)[b, 0, 0]
        corr = np.float32(-1e9) * (
            (mrow @ np.asarray(v, np.float32)[b]) @ np.asarray(Wv, np.float32)
            + mrow.sum() * np.asarray(bv, np.float32)
        )
        out[b] = acc + (corr @ np.asarray(Wo, np.float32) + np.asarray(bo, np.float32))
    att = attT.transpose(0, 1, 3, 2)
    return out, att


def kernel(q, k, v, mask, Wq, bq, Wk, bk, Wv, bv, Wo, bo):
    nc = get_nc()
    in_maps = make_in_maps(q, k, v, mask, Wq, bq, Wk, bk, Wv, bv, Wo, bo)
    res = run_bass_kernel_spmd(nc, in_maps, list(range(NCORES))).results
    return assemble(res, q, k, v, mask, Wv, bv, Wo, bo)


# revision 30
# speedup vs baseline: 1.0186x; 1.0186x over previous
"""Multi-head attention (B=2, S=2048, D=1024, 16 heads) on 8 Trainium2 cores.

Sharding: core c = 4*b + g handles batch b and heads [4g, 4g+4) — data
parallel over B, tensor parallel over heads (Wq/Wk/Wv column slices, Wo row
slices). The attention matrix never crosses cores.

Per-core device program (everything kept in transposed layouts so the PE
contracts over partitions and softmax denominators ride a ones-column):

  QhT/KhT [256, S] = Wslice.T @ x.T          (x.T streamed from DRAM)
  Vh      [S, 260] = x.T-chunks @ Wv slice   (natural layout, + ones col/head)
  scoresT [k, q]   = KhT-head.T-slices @ QhT-head-slices       (K=64)
  expT             = exp(scoresT / 8)        (ACT, psum->sbuf)
  ctxT_ext[65, q]  = [Vh_h | 1].T @ expT     (rows 0-63 ctx, row 64 = denom)
  rb[128, q]       = ones.T @ (1/denom)      (PE outer-product broadcast)
  attT             = expT * rb + (-1e9*mask)[k]  -> DMA out
  out_partial      = ctxT.T @ Wo_rows            -> DMA out

Matmuls run in float32r (hardware-rounded fp32: ~1.5e-4 rel err, 4x the
fp32 matmul rate). The host transposes attT back (stride view), sums the 4
partial outputs per batch, and adds the analytic mask-context correction
(-1e9 * mask @ (v@Wv + bv) @ Wo, constant over q) plus biases.
"""

import numpy as np

import concourse.bass as bass
import concourse.mybir as mybir
import concourse.tile as tile
from concourse import bacc
from concourse.bass_utils import run_bass_kernel_spmd
from concourse.masks import make_identity

B, S, D = 2, 2048, 1024
H_ALL, DEPTH = 16, 64
NCORES = 8
HG = 4                      # heads per core
GD = HG * DEPTH             # 256 projected dims per core
P = 128
NB = 512                    # q-block / matmul free-dim size
F32 = mybir.dt.float32
F32R = mybir.dt.float32r
AF = mybir.ActivationFunctionType

KC = D // P                 # 8 contraction chunks for projections
MQ = S // P                 # 16 seq chunks of 128
QB = S // NB                # 4 q-blocks of 512

_BUILT = {}
CFG = {}


def _build():
    nc = bacc.Bacc("TRN2", target_bir_lowering=False, debug=False)

    qT = nc.dram_tensor("qT", [D, S], F32, kind="ExternalInput").ap()
    kT = nc.dram_tensor("kT", [D, S], F32, kind="ExternalInput").ap()
    vT = nc.dram_tensor("vT", [D, S], F32, kind="ExternalInput").ap()
    wq = nc.dram_tensor("wq", [D, GD], F32, kind="ExternalInput").ap()
    wk = nc.dram_tensor("wk", [D, GD], F32, kind="ExternalInput").ap()
    wv = nc.dram_tensor("wv", [D, GD], F32, kind="ExternalInput").ap()
    wo = nc.dram_tensor("wo", [GD, D], F32, kind="ExternalInput").ap()
    bq = nc.dram_tensor("bq", [GD], F32, kind="ExternalInput").ap()
    bk = nc.dram_tensor("bk", [GD], F32, kind="ExternalInput").ap()
    bv = nc.dram_tensor("bv", [GD], F32, kind="ExternalInput").ap()
    mneg = nc.dram_tensor("mneg", [S], F32, kind="ExternalInput").ap()
    att_t = nc.dram_tensor("att_t", [HG, S, S], F32, kind="ExternalOutput").ap()
    out_p = nc.dram_tensor("out_p", [S, D], mybir.dt.bfloat16, kind="ExternalOutput").ap()

    with tile.TileContext(nc) as tc:
        _body(tc, qT, kT, vT, wq, wk, wv, wo, bq, bk, bv, mneg, att_t, out_p)
    nc.compile()
    return nc


def _body(tc, qT, kT, vT, wq, wk, wv, wo, bq, bk, bv, mneg, att_t, out_p):
    nc = tc.nc

    with tc.tile_pool(name="persist", bufs=1) as pp:
        # ---- persistent constants (loaded once) ----
        # weights, rearranged so contraction chunk kc sits at cols [kc*GD, ...)
        w_sb = {}
        with tc.tile_pool(name="wldp", bufs=2) as wldp:
            for nm, src in (("wq", wq), ("wk", wk), ("wv", wv)):
                t = pp.tile([P, KC * GD], F32R, tag=f"{nm}r", name=f"{nm}r")
                lt = wldp.tile([P, KC * GD], F32, tag="wld", name=f"{nm}ld")
                nc.sync.dma_start(
                    lt[:].rearrange("p (kc j) -> p kc j", kc=KC),
                    src.rearrange("(kc p) j -> p kc j", p=P),
                )
                nc.vector.tensor_copy(t[:], lt[:])
                w_sb[nm] = t
            wo_sb = pp.tile([P, 2 * D], F32R, tag="wor")
            wo_ld = wldp.tile([P, 2 * D], F32, tag="wld", name="wold")
            nc.sync.dma_start(
                wo_ld[:].rearrange("p (dc j) -> p dc j", dc=2),
                wo.rearrange("(dc p) j -> p dc j", p=P),
            )
            nc.vector.tensor_copy(wo_sb[:], wo_ld[:])

        bq_sb = pp.tile([P, 2], F32, tag="bq")
        nc.sync.dma_start(bq_sb[:], bq.rearrange("(m p) -> p m", p=P))
        bk_sb = pp.tile([P, 2], F32, tag="bk")
        nc.sync.dma_start(bk_sb[:], bk.rearrange("(m p) -> p m", p=P))
        bv_sb = pp.tile([P, 2], F32, tag="bv")
        nc.sync.dma_start(bv_sb[:], bv.rearrange("(m p) -> p m", p=P))
        mn_sb = pp.tile([P, MQ], F32, tag="mn")
        nc.sync.dma_start(mn_sb[:], mneg.rearrange("(a p) -> p a", p=P))
        onec_f = pp.tile([P, 1], F32, tag="onecf")
        nc.vector.memset(onec_f[:], 1.0)

        # ---- persistent activations ----
        QhT = [pp.tile([P, S], F32R, tag=f"QhT{m}", name=f"QhT{m}") for m in range(2)]
        KhT = [pp.tile([P, S], F32R, tag=f"KhT{m}", name=f"KhT{m}") for m in range(2)]
        Vh = [
            pp.tile([P, HG * 65], F32R, tag=f"Vh{m}", name=f"Vh{m}")
            for m in range(MQ)
        ]
        ctxT = [
            pp.tile([P, S], F32R, tag=f"ctxT{m}", name=f"ctxT{m}") for m in range(2)
        ]

        # ================= Phase A: projections =================
        with tc.tile_pool(name="stream", bufs=3) as sp, \
             tc.tile_pool(name="psA", bufs=1, space="PSUM") as psA:
            # V -> transposed proj (single pass over vT), then PE-transpose
            # into natural layout [S, GD] with a ones column per head
            VhT = [sp.tile([P, S], F32R, tag=f"VhT{m}", name=f"VhT{m}", bufs=1) for m in range(2)]
            psums = [
                psA.tile([P, NB], F32, tag=f"psA{i}", name=f"ps_wv_{i}")
                for i in range(8)
            ]
            for kc in range(KC):
                xs = sp.tile([P, S], F32, tag="xs")
                nc.sync.dma_start(xs[:], vT[kc * P : (kc + 1) * P, :])
                xr = sp.tile([P, S], F32R, tag="xr")
                nc.vector.tensor_copy(xr[:], xs[:])
                for m in range(2):
                    for n in range(QB):
                        nc.tensor.matmul(
                            psums[m * QB + n][:],
                            w_sb["wv"][:, kc * GD + m * P : kc * GD + (m + 1) * P],
                            xr[:, n * NB : (n + 1) * NB],
                            start=(kc == 0),
                            stop=(kc == KC - 1),
                        )
            for m in range(2):
                for n in range(QB):
                    nc.scalar.activation(
                        VhT[m][:, n * NB : (n + 1) * NB],
                        psums[m * QB + n][:],
                        AF.Identity,
                        bias=bv_sb[:, m : m + 1],
                    )
            ident_f = sp.tile([P, P], F32, tag="identf", bufs=1)
            make_identity(nc, ident_f[:])
            ident = sp.tile([P, P], F32R, tag="ident", bufs=1)
            nc.vector.tensor_copy(ident[:], ident_f[:])
            for m16 in range(MQ):
                for t2 in range(2):
                    pst = psA.tile(
                        [P, P], F32R, tag=f"psA{t2}", name=f"ps_tr_{m16}_{t2}"
                    )
                    nc.tensor.transpose(
                        pst[:], VhT[t2][:, m16 * P : (m16 + 1) * P], ident[:]
                    )
                    for hh in range(2):
                        h = 2 * t2 + hh
                        nc.scalar.activation(
                            Vh[m16][:, 65 * h : 65 * h + 64],
                            pst[:, 64 * hh : 64 * hh + 64],
                            AF.Identity,
                        )
                for h in range(HG):
                    nc.vector.tensor_copy(
                        Vh[m16][:, 65 * h + 64 : 65 * h + 65], onec_f[:]
                    )

            # Q and K -> transposed layout [GD, S]
            for src, wname, bias_sb, out_tiles in (
                (qT, "wq", bq_sb, QhT),
                (kT, "wk", bk_sb, KhT),
            ):
                psums = [
                    psA.tile([P, NB], F32, tag=f"psA{i}", name=f"ps_{wname}_{i}")
                    for i in range(8)
                ]
                for kc in range(KC):
                    xs = sp.tile([P, S], F32, tag="xs")
                    nc.sync.dma_start(xs[:], src[kc * P : (kc + 1) * P, :])
                    xr = sp.tile([P, S], F32R, tag="xr")
                    nc.vector.tensor_copy(xr[:], xs[:])
                    for m in range(2):
                        for n in range(QB):
                            nc.tensor.matmul(
                                psums[m * QB + n][:],
                                w_sb[wname][:, kc * GD + m * P : kc * GD + (m + 1) * P],
                                xr[:, n * NB : (n + 1) * NB],
                                start=(kc == 0),
                                stop=(kc == KC - 1),
                            )
                for m in range(2):
                    for n in range(QB):
                        nc.scalar.activation(
                            out_tiles[m][:, n * NB : (n + 1) * NB],
                            psums[m * QB + n][:],
                            AF.Identity,
                            bias=bias_sb[:, m : m + 1],
                        )
        # ================= Phase B: attention + fused output proj ==========
        # q-block outer, head inner: after all 4 heads finish a q-block its
        # ctxT columns are final, so the output projection for that q-range
        # runs inside phase B (overlapped). Normalize muls and mask adds are
        # split between GPSIMD (otherwise idle) and DVE.
        GP_MULS = CFG.get("gp_muls", 0)    # per block on gpsimd, rest DVE
        GP_MASKS = CFG.get("gp_masks", 12)  # per block on gpsimd, rest DVE
        with tc.tile_pool(name="attn", bufs=1) as ap_, \
             tc.tile_pool(name="outp", bufs=1) as op_, \
             tc.tile_pool(name="psB", bufs=1, space="PSUM") as psB:
            for qb in range(QB):
                qsl = slice(qb * NB, (qb + 1) * NB)
                for h in range(HG):
                    t2, o64 = h // 2, 64 * (h % 2)
                    psc = psB.tile([65, NB], F32, tag="psc", bufs=CFG.get("psc_bufs", 2), name=f"psc{h}_{qb}")
                    exps = []
                    for kc2 in range(MQ):
                        pss = psB.tile(
                            [P, NB], F32, tag="pss", bufs=CFG.get("pss_bufs", 4),
                            name=f"pss{h}_{qb}_{kc2}"
                        )
                        nc.tensor.matmul(
                            pss[:],
                            KhT[t2][o64 : o64 + 64, kc2 * P : (kc2 + 1) * P],
                            QhT[t2][o64 : o64 + 64, qsl],
                            start=True,
                            stop=True,
                        )
                        et = ap_.tile(
                            [P, NB], F32R, tag=f"exp{kc2}", bufs=CFG.get("exp3", 0) and (3 if kc2 < CFG.get("exp3", 0) else 2) or 2,
                            name=f"exp{h}_{qb}_{kc2}",
                        )
                        nc.scalar.activation(et[:], pss[:], AF.Exp, scale=0.125)
                        exps.append(et)
                        nc.tensor.matmul(
                            psc[:],
                            Vh[kc2][:, 65 * h : 65 * h + 65],
                            et[:],
                            start=(kc2 == 0),
                            stop=(kc2 == MQ - 1),
                        )
                    rec = ap_.tile([1, NB], F32R, tag="rec", bufs=2, name=f"rec{h}_{qb}")
                    with nc.allow_low_precision(reason="f32r ~ f32 for recip"):
                        nc.vector.reciprocal(rec[:], psc[64:65, :])
                    rb = ap_.tile([P, NB], F32R, tag="rb", bufs=CFG.get("rb_bufs", 2), name=f"rb{h}_{qb}")
                    nc.gpsimd.partition_broadcast(rb[:], rec[0:1, :])
                    nc.vector.tensor_mul(
                        ctxT[t2][o64 : o64 + 64, qsl], psc[0:64, :], rb[0:64, :]
                    )
                    for kc2 in range(MQ):
                        et = exps[kc2]
                        st = ap_.tile(
                            [P, NB], F32, tag="stage", bufs=CFG.get("stage_bufs", 12),
                            name=f"st{h}_{qb}_{kc2}",
                        )
                        mul_gp = kc2 < GP_MULS
                        mask_gp = GP_MULS <= kc2 < GP_MULS + GP_MASKS
                        if mul_gp:
                            nc.gpsimd.tensor_mul(st[:], et[:], rb[:])
                        else:
                            nc.vector.tensor_mul(st[:], et[:], rb[:])
                        if mask_gp:
                            nc.gpsimd.tensor_scalar_add(
                                st[:], st[:], mn_sb[:, kc2 : kc2 + 1]
                            )
                        else:
                            nc.vector.tensor_scalar_add(
                                st[:], st[:], mn_sb[:, kc2 : kc2 + 1]
                            )
                        nc.sync.dma_start(
                            att_t[h, kc2 * P : (kc2 + 1) * P, qsl], st[:]
                        )
                # output projection for this q-block (ctxT columns now final)
                for qc in range(4 * qb, 4 * (qb + 1)):
                    for nn in range(2):
                        pso = psB.tile(
                            [P, NB], F32, tag="pso", bufs=CFG.get("pso_bufs", 2), name=f"pso{qc}_{nn}"
                        )
                        for dc in range(2):
                            nc.tensor.matmul(
                                pso[:],
                                ctxT[dc][:, qc * P : (qc + 1) * P],
                                wo_sb[:, dc * D + nn * NB : dc * D + (nn + 1) * NB],
                                start=(dc == 0),
                                stop=(dc == 1),
                            )
                        ot = op_.tile([P, NB], mybir.dt.bfloat16, tag="ot", bufs=4, name=f"ot{qc}_{nn}")
                        nc.scalar.activation(ot[:], pso[:], AF.Identity)
                        nc.sync.dma_start(
                            out_p[qc * P : (qc + 1) * P, nn * NB : (nn + 1) * NB], ot[:]
                        )

def get_nc():
    if "nc" not in _BUILT:
        _BUILT["nc"] = _build()
    return _BUILT["nc"]


def make_in_maps(q, k, v, mask, Wq, bq, Wk, bk, Wv, bv, Wo, bo):
    q, k, v = (np.asarray(x, np.float32) for x in (q, k, v))
    in_maps = []
    for c in range(NCORES):
        b, g = divmod(c, HG)
        sl = slice(GD * g, GD * (g + 1))
        in_maps.append(
            {
                "qT": np.ascontiguousarray(q[b].T),
                "kT": np.ascontiguousarray(k[b].T),
                "vT": np.ascontiguousarray(v[b].T),
                "wq": np.ascontiguousarray(Wq[:, sl], dtype=np.float32),
                "wk": np.ascontiguousarray(Wk[:, sl], dtype=np.float32),
                "wv": np.ascontiguousarray(Wv[:, sl], dtype=np.float32),
                "wo": np.ascontiguousarray(Wo[sl, :], dtype=np.float32),
                "bq": np.ascontiguousarray(bq[sl], dtype=np.float32),
                "bk": np.ascontiguousarray(bk[sl], dtype=np.float32),
                "bv": np.ascontiguousarray(bv[sl], dtype=np.float32),
                "mneg": np.ascontiguousarray(
                    mask[b, 0, 0].astype(np.float32) * np.float32(-1e9)
                ),
            }
        )
    return in_maps


def assemble(results, q, k, v, mask, Wv, bv, Wo, bo):
    out = np.empty((B, S, D), np.float32)
    attT = np.empty((B, H_ALL, S, S), np.float32)
    for b in range(B):
        acc = None
        for g in range(HG):
            r = results[HG * b + g]
            op32 = np.asarray(r["out_p"], np.float32)
            acc = op32 if acc is None else acc + op32
            attT[b, HG * g : HG * (g + 1)] = r["att_t"]
        mrow = np.asarray(mask, np.float32)[b, 0, 0]
        corr = np.float32(-1e9) * (
            (mrow @ np.asarray(v, np.float32)[b]) @ np.asarray(Wv, np.float32)
            + mrow.sum() * np.asarray(bv, np.float32)
        )
        out[b] = acc + (corr @ np.asarray(Wo, np.float32) + np.asarray(bo, np.float32))
    att = attT.transpose(0, 1, 3, 2)
    return out, att


def kernel(q, k, v, mask, Wq, bq, Wk, bk, Wv, bv, Wo, bo):
    nc = get_nc()
    in_maps = make_in_maps(q, k, v, mask, Wq, bq, Wk, bk, Wv, bv, Wo, bo)
    res = run_bass_kernel_spmd(nc, in_maps, list(range(NCORES))).results
    return assemble(res, q, k, v, mask, Wv, bv, Wo, bo)


# revision 32
# speedup vs baseline: 1.0560x; 1.0367x over previous
"""Multi-head attention (B=2, S=2048, D=1024, 16 heads) on 8 Trainium2 cores.

Sharding: core c = 4*b + g handles batch b and heads [4g, 4g+4) — data
parallel over B, tensor parallel over heads (Wq/Wk/Wv column slices, Wo row
slices). The attention matrix never crosses cores.

Per-core device program (everything kept in transposed layouts so the PE
contracts over partitions and softmax denominators ride a ones-column):

  QhT/KhT [256, S] = Wslice.T @ x.T          (x.T streamed from DRAM)
  Vh      [S, 260] = x.T-chunks @ Wv slice   (natural layout, + ones col/head)
  scoresT [k, q]   = KhT-head.T-slices @ QhT-head-slices       (K=64)
  expT             = exp(scoresT / 8)        (ACT, psum->sbuf)
  ctxT_ext[65, q]  = [Vh_h | 1].T @ expT     (rows 0-63 ctx, row 64 = denom)
  rb[128, q]       = ones.T @ (1/denom)      (PE outer-product broadcast)
  attT             = expT * rb + (-1e9*mask)[k]  -> DMA out
  out_partial      = ctxT.T @ Wo_rows            -> DMA out

Matmuls run in float32r (hardware-rounded fp32: ~1.5e-4 rel err, 4x the
fp32 matmul rate). The host transposes attT back (stride view), sums the 4
partial outputs per batch, and adds the analytic mask-context correction
(-1e9 * mask @ (v@Wv + bv) @ Wo, constant over q) plus biases.
"""

import numpy as np

import concourse.bass as bass
import concourse.mybir as mybir
import concourse.tile as tile
from concourse import bacc
from concourse.bass_utils import run_bass_kernel_spmd
from concourse.masks import make_identity

B, S, D = 2, 2048, 1024
H_ALL, DEPTH = 16, 64
NCORES = 8
HG = 4                      # heads per core
GD = HG * DEPTH             # 256 projected dims per core
P = 128
NB = 512                    # q-block / matmul free-dim size
F32 = mybir.dt.float32
F32R = mybir.dt.float32r
AF = mybir.ActivationFunctionType

KC = D // P                 # 8 contraction chunks for projections
MQ = S // P                 # 16 seq chunks of 128
QB = S // NB                # 4 q-blocks of 512

_BUILT = {}
CFG = {}


def _build():
    nc = bacc.Bacc("TRN2", target_bir_lowering=False, debug=False)

    qT = nc.dram_tensor("qT", [D, S], F32, kind="ExternalInput").ap()
    kT = nc.dram_tensor("kT", [D, S], F32, kind="ExternalInput").ap()
    vT = nc.dram_tensor("vT", [D, S], F32, kind="ExternalInput").ap()
    wq = nc.dram_tensor("wq", [D, GD], F32, kind="ExternalInput").ap()
    wk = nc.dram_tensor("wk", [D, GD], F32, kind="ExternalInput").ap()
    wv = nc.dram_tensor("wv", [D, GD], F32, kind="ExternalInput").ap()
    wo = nc.dram_tensor("wo", [GD, D], F32, kind="ExternalInput").ap()
    bq = nc.dram_tensor("bq", [GD], F32, kind="ExternalInput").ap()
    bk = nc.dram_tensor("bk", [GD], F32, kind="ExternalInput").ap()
    bv = nc.dram_tensor("bv", [GD], F32, kind="ExternalInput").ap()
    mneg = nc.dram_tensor("mneg", [S], F32, kind="ExternalInput").ap()
    att_t = nc.dram_tensor("att_t", [HG, S, S], F32, kind="ExternalOutput").ap()
    out_p = nc.dram_tensor("out_p", [S, D], mybir.dt.bfloat16, kind="ExternalOutput").ap()

    with tile.TileContext(nc) as tc:
        _body(tc, qT, kT, vT, wq, wk, wv, wo, bq, bk, bv, mneg, att_t, out_p)
    nc.compile()
    return nc


def _body(tc, qT, kT, vT, wq, wk, wv, wo, bq, bk, bv, mneg, att_t, out_p):
    nc = tc.nc

    with tc.tile_pool(name="persist", bufs=1) as pp:
        # ---- persistent constants (loaded once) ----
        # weights, rearranged so contraction chunk kc sits at cols [kc*GD, ...)
        w_sb = {}
        with tc.tile_pool(name="wldp", bufs=2) as wldp:
            for nm, src in (("wq", wq), ("wk", wk), ("wv", wv)):
                t = pp.tile([P, KC * GD], F32R, tag=f"{nm}r", name=f"{nm}r")
                lt = wldp.tile([P, KC * GD], F32, tag="wld", name=f"{nm}ld")
                nc.sync.dma_start(
                    lt[:].rearrange("p (kc j) -> p kc j", kc=KC),
                    src.rearrange("(kc p) j -> p kc j", p=P),
                )
                nc.vector.tensor_copy(t[:], lt[:])
                w_sb[nm] = t
            wo_sb = pp.tile([P, 2 * D], F32R, tag="wor")
            wo_ld = wldp.tile([P, 2 * D], F32, tag="wld", name="wold")
            nc.sync.dma_start(
                wo_ld[:].rearrange("p (dc j) -> p dc j", dc=2),
                wo.rearrange("(dc p) j -> p dc j", p=P),
            )
            nc.vector.tensor_copy(wo_sb[:], wo_ld[:])

        bq_sb = pp.tile([P, 2], F32, tag="bq")
        nc.sync.dma_start(bq_sb[:], bq.rearrange("(m p) -> p m", p=P))
        bk_sb = pp.tile([P, 2], F32, tag="bk")
        nc.sync.dma_start(bk_sb[:], bk.rearrange("(m p) -> p m", p=P))
        bv_sb = pp.tile([P, 2], F32, tag="bv")
        nc.sync.dma_start(bv_sb[:], bv.rearrange("(m p) -> p m", p=P))
        mn_sb = pp.tile([P, MQ], F32, tag="mn")
        nc.sync.dma_start(mn_sb[:], mneg.rearrange("(a p) -> p a", p=P))
        onec_f = pp.tile([P, 1], F32, tag="onecf")
        nc.vector.memset(onec_f[:], 1.0)

        # ---- persistent activations ----
        QhT = [pp.tile([P, S], F32R, tag=f"QhT{m}", name=f"QhT{m}") for m in range(2)]
        KhT = [pp.tile([P, S], F32R, tag=f"KhT{m}", name=f"KhT{m}") for m in range(2)]
        Vh = [
            pp.tile([P, HG * 65], F32R, tag=f"Vh{m}", name=f"Vh{m}")
            for m in range(MQ)
        ]
        ctxT = [
            pp.tile([P, S], F32R, tag=f"ctxT{m}", name=f"ctxT{m}") for m in range(2)
        ]

        # ================= Phase A: projections =================
        with tc.tile_pool(name="stream", bufs=CFG.get("sbufs", 5)) as sp, \
             tc.tile_pool(name="psA", bufs=1, space="PSUM") as psA:
            # V -> transposed proj (single pass over vT), then PE-transpose
            # into natural layout [S, GD] with a ones column per head
            VhT = [sp.tile([P, S], F32R, tag=f"VhT{m}", name=f"VhT{m}", bufs=1) for m in range(2)]
            psums = [
                psA.tile([P, NB], F32, tag=f"psA{i}", name=f"ps_wv_{i}")
                for i in range(8)
            ]
            for kc in range(KC):
                xs = sp.tile([P, S], F32, tag="xs")
                nc.sync.dma_start(xs[:], vT[kc * P : (kc + 1) * P, :])
                xr = sp.tile([P, S], F32R, tag="xr")
                nc.vector.tensor_copy(xr[:], xs[:])
                for m in range(2):
                    for n in range(QB):
                        nc.tensor.matmul(
                            psums[m * QB + n][:],
                            w_sb["wv"][:, kc * GD + m * P : kc * GD + (m + 1) * P],
                            xr[:, n * NB : (n + 1) * NB],
                            start=(kc == 0),
                            stop=(kc == KC - 1),
                        )
            for m in range(2):
                for n in range(QB):
                    nc.scalar.activation(
                        VhT[m][:, n * NB : (n + 1) * NB],
                        psums[m * QB + n][:],
                        AF.Identity,
                        bias=bv_sb[:, m : m + 1],
                    )
            ident_f = sp.tile([P, P], F32, tag="identf", bufs=1)
            make_identity(nc, ident_f[:])
            ident = sp.tile([P, P], F32R, tag="ident", bufs=1)
            nc.vector.tensor_copy(ident[:], ident_f[:])
            for m16 in range(MQ):
                for t2 in range(2):
                    pst = psA.tile(
                        [P, P], F32R, tag=f"psA{t2}", name=f"ps_tr_{m16}_{t2}"
                    )
                    nc.tensor.transpose(
                        pst[:], VhT[t2][:, m16 * P : (m16 + 1) * P], ident[:]
                    )
                    for hh in range(2):
                        h = 2 * t2 + hh
                        nc.vector.tensor_copy(
                            Vh[m16][:, 65 * h : 65 * h + 64],
                            pst[:, 64 * hh : 64 * hh + 64],
                        )
                for h in range(HG):
                    nc.vector.tensor_copy(
                        Vh[m16][:, 65 * h + 64 : 65 * h + 65], onec_f[:]
                    )

            # Q and K -> transposed layout [GD, S]
            for src, wname, bias_sb, out_tiles in (
                (qT, "wq", bq_sb, QhT),
                (kT, "wk", bk_sb, KhT),
            ):
                psums = [
                    psA.tile([P, NB], F32, tag=f"psA{i}", name=f"ps_{wname}_{i}")
                    for i in range(8)
                ]
                for kc in range(KC):
                    xs = sp.tile([P, S], F32, tag="xs")
                    nc.sync.dma_start(xs[:], src[kc * P : (kc + 1) * P, :])
                    xr = sp.tile([P, S], F32R, tag="xr")
                    nc.vector.tensor_copy(xr[:], xs[:])
                    for m in range(2):
                        for n in range(QB):
                            nc.tensor.matmul(
                                psums[m * QB + n][:],
                                w_sb[wname][:, kc * GD + m * P : kc * GD + (m + 1) * P],
                                xr[:, n * NB : (n + 1) * NB],
                                start=(kc == 0),
                                stop=(kc == KC - 1),
                            )
                for m in range(2):
                    for n in range(QB):
                        nc.scalar.activation(
                            out_tiles[m][:, n * NB : (n + 1) * NB],
                            psums[m * QB + n][:],
                            AF.Identity,
                            bias=bias_sb[:, m : m + 1],
                        )
        # ================= Phase B: attention + fused output proj ==========
        # q-block outer, head inner: after all 4 heads finish a q-block its
        # ctxT columns are final, so the output projection for that q-range
        # runs inside phase B (overlapped). Normalize muls and mask adds are
        # split between GPSIMD (otherwise idle) and DVE.
        GP_MULS = CFG.get("gp_muls", 0)    # per block on gpsimd, rest DVE
        GP_MASKS = CFG.get("gp_masks", 12)  # per block on gpsimd, rest DVE
        with tc.tile_pool(name="attn", bufs=1) as ap_, \
             tc.tile_pool(name="outp", bufs=1) as op_, \
             tc.tile_pool(name="psB", bufs=1, space="PSUM") as psB:
            for qb in range(QB):
                qsl = slice(qb * NB, (qb + 1) * NB)
                for h in range(HG):
                    t2, o64 = h // 2, 64 * (h % 2)
                    psc = psB.tile([65, NB], F32, tag="psc", bufs=CFG.get("psc_bufs", 2), name=f"psc{h}_{qb}")
                    exps = []
                    for kc2 in range(MQ):
                        pss = psB.tile(
                            [P, NB], F32, tag="pss", bufs=CFG.get("pss_bufs", 4),
                            name=f"pss{h}_{qb}_{kc2}"
                        )
                        nc.tensor.matmul(
                            pss[:],
                            KhT[t2][o64 : o64 + 64, kc2 * P : (kc2 + 1) * P],
                            QhT[t2][o64 : o64 + 64, qsl],
                            start=True,
                            stop=True,
                        )
                        et = ap_.tile(
                            [P, NB], F32R, tag=f"exp{kc2}", bufs=CFG.get("exp3", 0) and (3 if kc2 < CFG.get("exp3", 0) else 2) or 2,
                            name=f"exp{h}_{qb}_{kc2}",
                        )
                        nc.scalar.activation(et[:], pss[:], AF.Exp, scale=0.125)
                        exps.append(et)
                        nc.tensor.matmul(
                            psc[:],
                            Vh[kc2][:, 65 * h : 65 * h + 65],
                            et[:],
                            start=(kc2 == 0),
                            stop=(kc2 == MQ - 1),
                        )
                    rec = ap_.tile([1, NB], F32R, tag="rec", bufs=2, name=f"rec{h}_{qb}")
                    with nc.allow_low_precision(reason="f32r ~ f32 for recip"):
                        nc.vector.reciprocal(rec[:], psc[64:65, :])
                    rb = ap_.tile([P, NB], F32R, tag="rb", bufs=CFG.get("rb_bufs", 2), name=f"rb{h}_{qb}")
                    nc.gpsimd.partition_broadcast(rb[:], rec[0:1, :])
                    nc.vector.tensor_mul(
                        ctxT[t2][o64 : o64 + 64, qsl], psc[0:64, :], rb[0:64, :]
                    )
                    for kc2 in range(MQ):
                        et = exps[kc2]
                        st = ap_.tile(
                            [P, NB], F32, tag="stage", bufs=CFG.get("stage_bufs", 12),
                            name=f"st{h}_{qb}_{kc2}",
                        )
                        mul_gp = kc2 < GP_MULS
                        mask_gp = GP_MULS <= kc2 < GP_MULS + GP_MASKS
                        if mul_gp:
                            nc.gpsimd.tensor_mul(st[:], et[:], rb[:])
                        else:
                            nc.vector.tensor_mul(st[:], et[:], rb[:])
                        if mask_gp:
                            nc.gpsimd.tensor_scalar_add(
                                st[:], st[:], mn_sb[:, kc2 : kc2 + 1]
                            )
                        else:
                            nc.vector.tensor_scalar_add(
                                st[:], st[:], mn_sb[:, kc2 : kc2 + 1]
                            )
                        nc.sync.dma_start(
                            att_t[h, kc2 * P : (kc2 + 1) * P, qsl], st[:]
                        )
                # output projection for this q-block (ctxT columns now final)
                for qc in range(4 * qb, 4 * (qb + 1)):
                    for nn in range(2):
                        pso = psB.tile(
                            [P, NB], F32, tag="pso", bufs=CFG.get("pso_bufs", 2), name=f"pso{qc}_{nn}"
                        )
                        for dc in range(2):
                            nc.tensor.matmul(
                                pso[:],
                                ctxT[dc][:, qc * P : (qc + 1) * P],
                                wo_sb[:, dc * D + nn * NB : dc * D + (nn + 1) * NB],
                                start=(dc == 0),
                                stop=(dc == 1),
                            )
                        ot = op_.tile([P, NB], mybir.dt.bfloat16, tag="ot", bufs=4, name=f"ot{qc}_{nn}")
                        nc.scalar.activation(ot[:], pso[:], AF.Identity)
                        nc.sync.dma_start(
                            out_p[qc * P : (qc + 1) * P, nn * NB : (nn + 1) * NB], ot[:]
                        )

def get_nc():
    if "nc" not in _BUILT:
        _BUILT["nc"] = _build()
    return _BUILT["nc"]


def make_in_maps(q, k, v, mask, Wq, bq, Wk, bk, Wv, bv, Wo, bo):
    q, k, v = (np.asarray(x, np.float32) for x in (q, k, v))
    in_maps = []
    for c in range(NCORES):
        b, g = divmod(c, HG)
        sl = slice(GD * g, GD * (g + 1))
        in_maps.append(
            {
                "qT": np.ascontiguousarray(q[b].T),
                "kT": np.ascontiguousarray(k[b].T),
                "vT": np.ascontiguousarray(v[b].T),
                "wq": np.ascontiguousarray(Wq[:, sl], dtype=np.float32),
                "wk": np.ascontiguousarray(Wk[:, sl], dtype=np.float32),
                "wv": np.ascontiguousarray(Wv[:, sl], dtype=np.float32),
                "wo": np.ascontiguousarray(Wo[sl, :], dtype=np.float32),
                "bq": np.ascontiguousarray(bq[sl], dtype=np.float32),
                "bk": np.ascontiguousarray(bk[sl], dtype=np.float32),
                "bv": np.ascontiguousarray(bv[sl], dtype=np.float32),
                "mneg": np.ascontiguousarray(
                    mask[b, 0, 0].astype(np.float32) * np.float32(-1e9)
                ),
            }
        )
    return in_maps


def assemble(results, q, k, v, mask, Wv, bv, Wo, bo):
    out = np.empty((B, S, D), np.float32)
    attT = np.empty((B, H_ALL, S, S), np.float32)
    for b in range(B):
        acc = None
        for g in range(HG):
            r = results[HG * b + g]
            op32 = np.asarray(r["out_p"], np.float32)
            acc = op32 if acc is None else acc + op32
            attT[b, HG * g : HG * (g + 1)] = r["att_t"]
        mrow = np.asarray(mask, np.float32)[b, 0, 0]
        corr = np.float32(-1e9) * (
            (mrow @ np.asarray(v, np.float32)[b]) @ np.asarray(Wv, np.float32)
            + mrow.sum() * np.asarray(bv, np.float32)
        )
        out[b] = acc + (corr @ np.asarray(Wo, np.float32) + np.asarray(bo, np.float32))
    att = attT.transpose(0, 1, 3, 2)
    return out, att


def kernel(q, k, v, mask, Wq, bq, Wk, bk, Wv, bv, Wo, bo):
    nc = get_nc()
    in_maps = make_in_maps(q, k, v, mask, Wq, bq, Wk, bk, Wv, bv, Wo, bo)
    res = run_bass_kernel_spmd(nc, in_maps, list(range(NCORES))).results
    return assemble(res, q, k, v, mask, Wv, bv, Wo, bo)
